# revision 17
# baseline (speedup 1.0000x reference)
"""Fused single-head attention (QKV proj + softmax*scale + AV) on 8 trn2 cores.

Reference computation (fp32):
    qkv = x @ W.T            x:[4,4096,768]  W:[192,768]
    q,k,v = split(qkv, 64)
    A = q @ k.T              (no pre-softmax scale)
    out = softmax(A) / 8 @ v

Sharding: core c handles batch b=c//2, query half qh=c%2 (2048 queries),
full 4096 keys of that batch. SPMD-uniform program: the host rolls the
key/value columns of x^T by qh*2048 so every core's own queries are
always columns 0:2048 (softmax is permutation-invariant over keys).

Design notes (from perfetto traces; see git-less history in transcript):
  - The PE serializes ALL matmuls (no row-group concurrency) at 1 col/
    cycle, max N=512/matmul (ISA s3d3_mm_num_elements): the kernel is
    column-stream bound: proj 36864 + scores 65536 + AV 65536 cols.
  - Inputs ship as fp16 (6.3 MB vs 12.6 fp32; DMA would otherwise
    starve the PE ~25us). fp16 passes precision (5.9e-3 vs the 2e-2
    gate); bf16 inputs FAIL (2.7e-2).
  - exp on ACT only (1 elem/lane/cycle, ~70us busy, just under PE's
    ~94us), bf16 out, constant bias -40 (softmax-invariant; row max
    never needed). P/V stay bf16 (fp16 lacks exp range; fp8 fails
    precision).
  - Weight loads (LDWEIGHTS) are unhidden in this toolchain
    (--enable-ldw-opt=false), ~110-140ns per swap: all stationary
    operands are padded to 128 columns so the compiler-automatic FWL
    (fast weight load) halves them - v_aug is [128k, 128] with col 64
    = ones (rowsum) and 65: = zeros; the Q weight chunk is zero-padded
    to 128.
  - Input DMA: wt on the ACT hwdge queue, xt on SP as 2-chunk-pair
    DMAs in consumption order (SP issue is ~610ns/instr, so 1-chunk
    DMAs are issue-bound; multi-queue xt splits let later sbs steal
    HBM bandwidth from sb0 and starve the ramp - measured worse).
  - PE warms up on identb (gpsimd-built, no DMA dep) to open the HAM
    clock-gate (cold PE = half speed; a ramp gap resets the 3us warm
    window).

Per-core dataflow, 1024-wide super-blocks (NSB=4):
    proj: per sb a 6-chunk fp16 chain -> [K^T|V^T] and (sb<2) Q^T.
    kt [64,4096] f16, qt [64,2048] f16, vt bf16 -> PE-transposed into
    natural V tiles v_sb. Slots (h,t): scores at[128k,1024q] =
    kt_t^T.qt_h -> ACT exp -> P^T bf16 -> AV acc[128,1024] +=
    v_aug^T.P^T accumulated over t=0..31 (rowsum lands in row 64).
    Finalize: PE-transpose acc_sb 128-q blocks (batched, one psum tile
    + one batched reciprocal), out = ot * recip(rowsum) * 0.125.

Scheduling: proj chains, V transposes and finalizes interleave into
the slot stream as PE filler (PSUM budget 16KB/partition: 3 rotating
[128,1024] fp32 tiles + one [128,1024] acc); AV of slot i-1 is emitted
after scores of slot i so the PE never sits directly behind the ACT
exp; the last AV is split 2x512 with 256-col acc staging so tail
finalizes start early.
"""

import sys

import numpy as np

for _p in ("/opt/trn_rl_repo",):
    if _p not in sys.path:
        sys.path.insert(0, _p)

import concourse.mybir as mybir  # noqa: E402
import concourse.tile as tile  # noqa: E402
from concourse import bacc  # noqa: E402
from concourse.bass_utils import run_bass_kernel_spmd  # noqa: E402
from concourse.masks import make_identity  # noqa: E402

B, S, D, DH = 4, 4096, 768, 64
QN = S // 2          # queries per core
NSB = 4              # 1024-wide super-blocks of s
SBW = 1024
NKT = 32             # 128-wide key tiles
HALF = 1024          # q-half for the slot loop
EXP_BIAS = -40.0     # global score offset (softmax-invariant), fp32 headroom
MMW = 512            # matmul rhs width (hard ISA cap: N<=512 per matmul)

F32 = mybir.dt.float32
F16 = mybir.dt.float16
BF16 = mybir.dt.bfloat16

_NC_CACHE = None
LAST_RESULTS = None


def _build():
    nc = bacc.Bacc(num_devices=8)
    xt_d = nc.dram_tensor("xt", [D, S], F16, kind="ExternalInput")
    # wt cols: 6 contraction chunks x [K|V (128) | Q (64) | zero pad (64)]
    wt_d = nc.dram_tensor("wt", [128, 6 * 256], F16, kind="ExternalInput")
    out_d = nc.dram_tensor("out", [QN, DH], F32, kind="ExternalOutput")

    with tile.TileContext(nc) as tc:
        with (
            tc.tile_pool(name="big", bufs=1) as big,
            tc.tile_pool(name="psmm", bufs=3, space="PSUM") as psmm,
            tc.tile_pool(name="psacc", bufs=1, space="PSUM") as psacc,
            tc.tile_pool(name="pt", bufs=6) as ptp,
            tc.tile_pool(name="small", bufs=4) as small,
        ):
            xt_tiles = [
                big.tile([128, 6, SBW], F16, tag=f"xt{sb}", name=f"xt{sb}")
                for sb in range(NSB)
            ]
            wt_sb = big.tile([128, 6 * 256], F16)
            kt = big.tile([64, S], F16)
            qt = big.tile([64, QN], F16)
            vt = big.tile([64, S], BF16)
            v_sb = big.tile([128, NKT, 128], BF16)  # 0:64=V, 64=ones, 65:=0
            # cols 65:128 are zero padding so the AV lhsT is a full 128-col
            # weight load (FWL, ~2x faster; 65-col loads can't FWL)
            acc_sb = big.tile([65, QN], F32)
            osb = big.tile([128, 16, DH], F32)
            ident = big.tile([128, 128], F32)
            identb = big.tile([128, 128], BF16)
            ebias = big.tile([128, 1], F32)
            escr = big.tile([128, 1], F32)

            # ---- input DMA split across issue queues: SP (hwdge) takes wt
            # + sb0 chunk-granular; ACT (hwdge) sb1; Pool (swdge on the idle
            # q7s) sb2+sb3. SP alone needs ~610ns per issue, serializing the
            # whole input stream ~15us; the split gets every sb moving early.
            def _xt_src(sb):
                return xt_d[:, sb * SBW:(sb + 1) * SBW].rearrange(
                    "(k p) s -> p k s", p=128
                )

            # wt rides the ACT hwdge queue so SP's first issue is already
            # sb0 chunk 0; all xt stays on SP IN CONSUMPTION ORDER (a
            # multi-queue split lets sb2/3 steal HBM bandwidth from the
            # sb0/sb1 transfers the PE needs first).
            nc.scalar.dma_start(out=wt_sb[:], in_=wt_d[:])
            for sb in (0, 1, 2, 3):
                for k0 in range(0, 6, 2):
                    nc.sync.dma_start(
                        out=xt_tiles[sb][:, k0:k0 + 2, :],
                        in_=_xt_src(sb)[:, k0:k0 + 2, :],
                    )

            # ---- cheap setup off the PE
            nc.vector.memset(ebias[:], EXP_BIAS)
            nc.vector.memset(v_sb[:, :, 64:128], 0.0)
            nc.vector.memset(v_sb[:, :, 64:65], 1.0)
            make_identity(nc, identb[:])
            make_identity(nc, ident[:])
            # prime the ACT exp table set during the DMA window
            nc.scalar.activation(
                out=escr[:], in_=ebias[:],
                func=mybir.ActivationFunctionType.Exp, bias=ebias[:],
            )

            # ---- PE warmup, DMA-independent (opens the HAM clock-gate; a
            # ramp gap resets the 3us warm-up window, so bridge until sb0
            # chunks flow). identb comes from gpsimd, up ~3us before DVE.
            wps = psmm.tile([128, HALF], F32, tag="mm")
            for _w in range(16):
                nc.tensor.matmul(
                    wps[:, 512 * (_w % 2):512 * (_w % 2) + 128],
                    identb[:, 0:128], identb[:],
                    start=True, stop=True,
                )

            # ---- emission helpers --------------------------------------
            kv_state = {}

            def emit_kv(sb, k0, k1):
                """Chunks k0:k1 of sb's K/V projection chain."""
                if k0 == 0:
                    kv_state[sb] = psmm.tile(
                        [128, HALF], F32, tag="mm", name=f"kv_ps{sb}"
                    )
                ps = kv_state[sb]
                for k in range(k0, k1):
                    for c in range(HALF // MMW):
                        csl = slice(c * MMW, (c + 1) * MMW)
                        nc.tensor.matmul(
                            ps[:, csl],
                            wt_sb[:, k * 256:k * 256 + 128],
                            xt_tiles[sb][:, k, c * MMW:(c + 1) * MMW],
                            start=(k == 0), stop=(k == 5),
                        )
                if k1 == 6:
                    s0 = sb * SBW
                    nc.vector.tensor_copy(kt[:, s0:s0 + 512], ps[0:64, 0:512])
                    nc.vector.tensor_copy(
                        kt[:, s0 + 512:s0 + SBW], ps[0:64, 512:SBW]
                    )
                    nc.vector.tensor_copy(
                        vt[:, s0:s0 + 512], ps[64:128, 0:512]
                    )
                    nc.vector.tensor_copy(
                        vt[:, s0 + 512:s0 + SBW], ps[64:128, 512:SBW]
                    )

            def emit_q(sb, k0, k1):
                """Chunks k0:k1 of sb's Q projection chain (sb 0 or 1)."""
                if k0 == 0:
                    kv_state[("q", sb)] = psmm.tile(
                        [128, HALF], F32, tag="mm", name=f"q_ps{sb}"
                    )
                ps = kv_state[("q", sb)]
                for k in range(k0, k1):
                    for c in range(HALF // MMW):
                        csl = slice(c * MMW, (c + 1) * MMW)
                        nc.tensor.matmul(
                            ps[:, csl],
                            wt_sb[:, k * 256 + 128:(k + 1) * 256],
                            xt_tiles[sb][:, k, c * MMW:(c + 1) * MMW],
                            start=(k == 0), stop=(k == 5),
                        )
                if k1 == 6:
                    nc.vector.tensor_copy(
                        qt[:, sb * SBW:(sb + 1) * SBW], ps[0:64, :]
                    )
                    kv_state.pop(("q", sb))

            def emit_vtrans(sb, i0, i1):
                """V natural tiles i0:i1 (of 8) for sb via PE transpose into
                the spent kv_ps tile (bf16 view; WAR on the kt/vt copies)."""
                ps16 = kv_state[sb][:].bitcast(BF16)
                for i in range(i0, i1):
                    t = sb * 8 + i
                    tsl = slice(i * 64, (i + 1) * 64)
                    nc.tensor.transpose(
                        ps16[:, tsl],
                        vt[:, t * 128:(t + 1) * 128],
                        identb[0:64, 0:64],
                    )
                    nc.vector.tensor_copy(v_sb[:, t, 0:64], ps16[:, tsl])
                if i1 == 8:
                    kv_state.pop(sb)

            def emit_vtrans_dma(sb):
                """V natural tiles for sb via the DMA xbar transpose engine
                (SP hwdge queue, idle after input issue). Only for sb>=2:
                earlier tiles are needed before the SP queue drains."""
                for i in range(8):
                    t = sb * 8 + i
                    nc.sync.dma_start_transpose(
                        out=v_sb[:, t, 0:64],
                        in_=vt[:, t * 128:(t + 1) * 128],
                    )
                kv_state.pop(sb, None)

            ats = {}
            pts = {}
            accs = {}

            def emit_scores(h, t):
                at = psmm.tile([128, HALF], F32, tag="mm")
                for c in range(HALF // MMW):
                    csl = slice(c * MMW, (c + 1) * MMW)
                    qsl = slice(h * HALF + c * MMW, h * HALF + (c + 1) * MMW)
                    nc.tensor.matmul(
                        at[:, csl],
                        kt[:, t * 128:(t + 1) * 128],
                        qt[:, qsl],
                        start=True, stop=True,
                    )
                pt = ptp.tile([128, HALF], BF16, tag="pt")
                if (h, t) == (1, NKT - 1):
                    for c in range(2):
                        esl = slice(c * 512, (c + 1) * 512)
                        nc.scalar.activation(
                            out=pt[:, esl], in_=at[:, esl],
                            func=mybir.ActivationFunctionType.Exp,
                            bias=ebias[:],
                        )
                else:
                    nc.scalar.activation(
                        out=pt[:], in_=at[:],
                        func=mybir.ActivationFunctionType.Exp, bias=ebias[:],
                    )
                pts[(h, t)] = pt

            def emit_av(h, t):
                pt = pts.pop((h, t))
                acc = accs[h]
                for c in range(HALF // MMW):
                    csl = slice(c * MMW, (c + 1) * MMW)
                    nc.tensor.matmul(
                        acc[:, csl],
                        v_sb[:, t, :],
                        pt[:, csl],
                        start=(t == 0), stop=(t == NKT - 1),
                        skip_group_check=True,
                    )

            def emit_out_dma(oc):
                nc.sync.dma_start(
                    out=out_d[:].rearrange("(t p) d -> p t d", p=128)[
                        :, 4 * oc:4 * (oc + 1), :],
                    in_=osb[:, 4 * oc:4 * (oc + 1), :],
                )

            def emit_fin(gblks):
                """Batch: transpose each 128-q block into one psum tile, one
                batched reciprocal, then per-block scale (avoids the PE-DVE
                ping-pong that stalled the tail)."""
                n = len(gblks)
                ot = psmm.tile([128, HALF], F32, tag="mm")
                for j, g in enumerate(gblks):
                    nc.tensor.transpose(
                        ot[:, 128 * j:128 * j + 65],
                        acc_sb[:, g * 128:(g + 1) * 128],
                        ident[0:65, 0:65],
                    )
                r = small.tile([128, 4], F32, tag="r")
                nc.vector.reciprocal(
                    r[:, 0:n],
                    ot[:].rearrange("p (j c) -> p j c", c=128)[:, 0:n, 64:65],
                )
                for j, g in enumerate(gblks):
                    nc.vector.tensor_scalar(
                        osb[:, g, :], ot[:, 128 * j:128 * j + 64], r[:, j:j + 1],
                        0.125,
                        op0=mybir.AluOpType.mult, op1=mybir.AluOpType.mult,
                    )

            # ---- pre-slot ramp: sb0 proj (kv then q), then V tiles 0:8 ----
            emit_kv(0, 0, 2)
            emit_kv(0, 2, 4)
            emit_kv(0, 4, 6)
            emit_q(0, 0, 6)
            emit_vtrans(0, 0, 8)
            accs[0] = psacc.tile([128, HALF], F32, tag="acc", name="acc0")

            # ---- slot stream -------------------------------------------
            # filler[i]: PE work emitted right after scores of slot i.
            # kv_sb must land before its slots (kt t: sb1 -> slots 8.., etc);
            # Vtrans sb before its AV slots; q1 before slot 32; all proj
            # windows sit in h0 so h1 slots have a free psmm buf for fins.
            filler = {
                2: [("kv", 1, 0, 3)], 3: [("kv", 1, 3, 6)],
                4: [("vt", 1, 0, 2)], 5: [("vt", 1, 2, 4)],
                6: [("vt", 1, 4, 6), ("kv", 2, 0, 3)],
                7: [("kv", 2, 3, 6), ("vt", 1, 6, 8)],
                8: [("vtd", 2)],
                12: [("kv", 3, 0, 3)], 13: [("kv", 3, 3, 6)],
                14: [("vtd", 3)],
                21: [("q", 1, 0, 2)], 22: [("q", 1, 2, 4)], 23: [("q", 1, 4, 6)],
                # h1: finalize h0 blocks two per slot-pair; out DMAs staggered
                34: [("fin", (0, 1))], 36: [("fin", (2, 3))],
                38: [("fin", (4, 5))], 39: [("odma", 0)],
                40: [("fin", (6, 7))], 41: [("odma", 1)],
            }

            slots = [(0, t) for t in range(NKT)] + [(1, t) for t in range(NKT)]
            for i, (h, t) in enumerate(slots):
                emit_scores(h, t)
                for f in filler.get(i, ()):
                    if f[0] == "kv":
                        emit_kv(f[1], f[2], f[3])
                    elif f[0] == "q":
                        emit_q(f[1], f[2], f[3])
                    elif f[0] == "vt":
                        emit_vtrans(f[1], f[2], f[3])
                    elif f[0] == "vtd":
                        emit_vtrans_dma(f[1])
                    elif f[0] == "fin":
                        emit_fin(f[1])
                    elif f[0] == "odma":
                        emit_out_dma(f[1])
                if i > 0:
                    ph, pt_ = slots[i - 1]
                    emit_av(ph, pt_)
                    if (ph, pt_) == (0, NKT - 1):
                        # h0 accumulation complete: stage and swap acc
                        nc.vector.tensor_copy(
                            acc_sb[:, 0:512], accs[0][0:65, 0:512]
                        )
                        nc.vector.tensor_copy(
                            acc_sb[:, 512:HALF], accs[0][0:65, 512:HALF]
                        )
                        accs[1] = psacc.tile(
                            [128, HALF], F32, tag="acc", name="acc1"
                        )

            # ---- tail: last AV split so each 512-q chunk stages early ----
            fpt = pts.pop((1, NKT - 1))
            for c in range(2):
                csl = slice(c * 512, (c + 1) * 512)
                nc.tensor.matmul(
                    accs[1][:, csl], v_sb[:, NKT - 1, :], fpt[:, csl],
                    start=False, stop=True, skip_group_check=True,
                )
                for cc in range(2):
                    o = c * 512 + cc * 256
                    nc.vector.tensor_copy(
                        acc_sb[:, HALF + o:HALF + o + 256],
                        accs[1][0:65, c * 512 + cc * 256:c * 512 + (cc + 1) * 256],
                    )
                    g0 = 8 + 2 * (2 * c + cc)
                    emit_fin((g0, g0 + 1))
                emit_out_dma(2 + c)

    nc.finalize()
    return nc


def _get_nc():
    global _NC_CACHE
    if _NC_CACHE is None:
        _NC_CACHE = _build()
    return _NC_CACHE


def kernel(x, W, _trace=False):
    global LAST_RESULTS
    x = np.ascontiguousarray(np.asarray(x), dtype=np.float32)
    W = np.ascontiguousarray(np.asarray(W), dtype=np.float32)
    assert x.shape == (B, S, D) and W.shape == (3 * DH, D)

    # wt cols per chunk k: [K rows | V rows | Q rows | zero pad]
    wtf = np.concatenate(
        [W[DH:2 * DH], W[2 * DH:], W[:DH], np.zeros((DH, D), np.float32)],
        axis=0,
    ).T
    wt = np.ascontiguousarray(
        wtf.reshape(6, 128, 256).transpose(1, 0, 2).reshape(128, 6 * 256)
    ).astype(np.float16)

    in_maps = []
    for c in range(8):
        b, qh = divmod(c, 2)
        xtb = x[b].T.astype(np.float16)  # [768, 4096]
        if qh:
            xtc = np.ascontiguousarray(
                np.concatenate([xtb[:, QN:], xtb[:, :QN]], axis=1)
            )
        else:
            xtc = np.ascontiguousarray(xtb)
        in_maps.append({"xt": xtc, "wt": wt})

    nc = _get_nc()
    res = run_bass_kernel_spmd(nc, in_maps, list(range(8)), trace=_trace)
    LAST_RESULTS = res

    out = np.empty((B, S, DH), np.float32)
    for c in range(8):
        b, qh = divmod(c, 2)
        out[b, qh * QN:(qh + 1) * QN] = res.results[c]["out"]
    return out


# revision 18
# speedup vs baseline: 1.0215x; 1.0215x over previous
"""Fused single-head attention (QKV proj + softmax*scale + AV) on 8 trn2 cores.

Reference computation (fp32):
    qkv = x @ W.T            x:[4,4096,768]  W:[192,768]
    q,k,v = split(qkv, 64)
    A = q @ k.T              (no pre-softmax scale)
    out = softmax(A) / 8 @ v

Sharding: core c handles batch b=c//2, query half qh=c%2 (2048 queries),
full 4096 keys of that batch. SPMD-uniform program: the host rolls the
key/value columns of x^T by qh*2048 so every core's own queries are
always columns 0:2048 (softmax is permutation-invariant over keys).

Design notes (from perfetto traces; see git-less history in transcript):
  - The PE serializes ALL matmuls (no row-group concurrency) at 1 col/
    cycle, max N=512/matmul (ISA s3d3_mm_num_elements): the kernel is
    column-stream bound: proj 36864 + scores 65536 + AV 65536 cols.
  - Inputs ship as fp16 (6.3 MB vs 12.6 fp32; DMA would otherwise
    starve the PE ~25us). fp16 passes precision (5.9e-3 vs the 2e-2
    gate); bf16 inputs FAIL (2.7e-2).
  - exp on ACT only (1 elem/lane/cycle, ~70us busy, just under PE's
    ~94us), bf16 out, constant bias -40 (softmax-invariant; row max
    never needed). P/V stay bf16 (fp16 lacks exp range; fp8 fails
    precision).
  - Weight loads (LDWEIGHTS) are unhidden in this toolchain
    (--enable-ldw-opt=false), ~110-140ns per swap: all stationary
    operands are padded to 128 columns so the compiler-automatic FWL
    (fast weight load) halves them - v_aug is [128k, 128] with col 64
    = ones (rowsum) and 65: = zeros; the Q weight chunk is zero-padded
    to 128.
  - Input DMA: wt on the ACT hwdge queue, xt on SP as 2-chunk-pair
    DMAs in consumption order (SP issue is ~610ns/instr, so 1-chunk
    DMAs are issue-bound; multi-queue xt splits let later sbs steal
    HBM bandwidth from sb0 and starve the ramp - measured worse).
  - PE warms up on identb (gpsimd-built, no DMA dep) to open the HAM
    clock-gate (cold PE = half speed; a ramp gap resets the 3us warm
    window).

Per-core dataflow, 1024-wide super-blocks (NSB=4):
    proj: per sb a 6-chunk fp16 chain -> [K^T|V^T] and (sb<2) Q^T.
    kt [64,4096] f16, qt [64,2048] f16, vt bf16 -> PE-transposed into
    natural V tiles v_sb. Slots (h,t): scores at[128k,1024q] =
    kt_t^T.qt_h -> ACT exp -> P^T bf16 -> AV acc[128,1024] +=
    v_aug^T.P^T accumulated over t=0..31 (rowsum lands in row 64).
    Finalize: PE-transpose acc_sb 128-q blocks (batched, one psum tile
    + one batched reciprocal), out = ot * recip(rowsum) * 0.125.

Scheduling: proj chains, V transposes and finalizes interleave into
the slot stream as PE filler (PSUM budget 16KB/partition: 3 rotating
[128,1024] fp32 tiles + one [128,1024] acc); AV of slot i-1 is emitted
after scores of slot i so the PE never sits directly behind the ACT
exp; the last AV is split 2x512 with 256-col acc staging so tail
finalizes start early.
"""

import sys

import numpy as np

for _p in ("/opt/trn_rl_repo",):
    if _p not in sys.path:
        sys.path.insert(0, _p)

import concourse.mybir as mybir  # noqa: E402
import concourse.tile as tile  # noqa: E402
from concourse import bacc  # noqa: E402
from concourse.bass_utils import run_bass_kernel_spmd  # noqa: E402
from concourse.masks import make_identity  # noqa: E402

B, S, D, DH = 4, 4096, 768, 64
QN = S // 2          # queries per core
NSB = 4              # 1024-wide super-blocks of s
SBW = 1024
NKT = 32             # 128-wide key tiles
HALF = 1024          # q-half for the slot loop
EXP_BIAS = -40.0     # global score offset (softmax-invariant), fp32 headroom
MMW = 512            # matmul rhs width (hard ISA cap: N<=512 per matmul)

F32 = mybir.dt.float32
F16 = mybir.dt.float16
BF16 = mybir.dt.bfloat16

_NC_CACHE = None
LAST_RESULTS = None


def _build():
    nc = bacc.Bacc(num_devices=8)
    xt_d = nc.dram_tensor("xt", [D, S], F16, kind="ExternalInput")
    # wt cols: 6 contraction chunks x [K|V (128) | Q (64) | zero pad (64)]
    wt_d = nc.dram_tensor("wt", [128, 6 * 256], F16, kind="ExternalInput")
    out_d = nc.dram_tensor("out", [QN, DH], F32, kind="ExternalOutput")

    with tile.TileContext(nc) as tc:
        with (
            tc.tile_pool(name="big", bufs=1) as big,
            tc.tile_pool(name="psmm", bufs=3, space="PSUM") as psmm,
            tc.tile_pool(name="psacc", bufs=1, space="PSUM") as psacc,
            tc.tile_pool(name="pt", bufs=6) as ptp,
            tc.tile_pool(name="small", bufs=4) as small,
        ):
            xt_tiles = [
                big.tile([128, 6, SBW], F16, tag=f"xt{sb}", name=f"xt{sb}")
                for sb in range(NSB)
            ]
            wt_sb = big.tile([128, 6 * 256], F16)
            kt = big.tile([64, S], F16)
            qt = big.tile([64, QN], F16)
            vt = big.tile([64, S], BF16)
            v_sb = big.tile([128, NKT, 128], BF16)  # 0:64=V, 64=ones, 65:=0
            # cols 65:128 are zero padding so the AV lhsT is a full 128-col
            # weight load (FWL, ~2x faster; 65-col loads can't FWL)
            acc_sb = big.tile([65, QN], F32)
            osb = big.tile([128, 16, DH], F32)
            ident = big.tile([128, 128], F32)
            identb = big.tile([128, 128], BF16)
            ebias = big.tile([128, 1], F32)
            escr = big.tile([128, 1], F32)

            # ---- input DMA split across issue queues: SP (hwdge) takes wt
            # + sb0 chunk-granular; ACT (hwdge) sb1; Pool (swdge on the idle
            # q7s) sb2+sb3. SP alone needs ~610ns per issue, serializing the
            # whole input stream ~15us; the split gets every sb moving early.
            def _xt_src(sb):
                return xt_d[:, sb * SBW:(sb + 1) * SBW].rearrange(
                    "(k p) s -> p k s", p=128
                )

            # wt rides the ACT hwdge queue so SP's first issue is already
            # sb0 chunk 0; all xt stays on SP IN CONSUMPTION ORDER (a
            # multi-queue split lets sb2/3 steal HBM bandwidth from the
            # sb0/sb1 transfers the PE needs first).
            nc.scalar.dma_start(out=wt_sb[:], in_=wt_d[:])
            for sb in (0, 1, 2, 3):
                for k0 in range(0, 6, 2):
                    nc.sync.dma_start(
                        out=xt_tiles[sb][:, k0:k0 + 2, :],
                        in_=_xt_src(sb)[:, k0:k0 + 2, :],
                    )

            # ---- cheap setup off the PE
            nc.vector.memset(ebias[:], EXP_BIAS)
            nc.vector.memset(v_sb[:, :, 64:128], 0.0)
            nc.vector.memset(v_sb[:, :, 64:65], 1.0)
            make_identity(nc, identb[:])
            make_identity(nc, ident[:])
            # prime the ACT exp table set during the DMA window
            nc.scalar.activation(
                out=escr[:], in_=ebias[:],
                func=mybir.ActivationFunctionType.Exp, bias=ebias[:],
            )

            # ---- PE warmup, DMA-independent (opens the HAM clock-gate; a
            # ramp gap resets the 3us warm-up window, so bridge until sb0
            # chunks flow). identb comes from gpsimd, up ~3us before DVE.
            wps = psmm.tile([128, HALF], F32, tag="mm")
            for _w in range(16):
                nc.tensor.matmul(
                    wps[:, 512 * (_w % 2):512 * (_w % 2) + 128],
                    identb[:, 0:128], identb[:],
                    start=True, stop=True,
                )

            # ---- emission helpers --------------------------------------
            kv_state = {}

            def emit_kv(sb, k0, k1):
                """Chunks k0:k1 of sb's K/V projection chain."""
                if k0 == 0:
                    kv_state[sb] = psmm.tile(
                        [128, HALF], F32, tag="mm", name=f"kv_ps{sb}"
                    )
                ps = kv_state[sb]
                for k in range(k0, k1):
                    for c in range(HALF // MMW):
                        csl = slice(c * MMW, (c + 1) * MMW)
                        nc.tensor.matmul(
                            ps[:, csl],
                            wt_sb[:, k * 256:k * 256 + 128],
                            xt_tiles[sb][:, k, c * MMW:(c + 1) * MMW],
                            start=(k == 0), stop=(k == 5),
                        )
                if k1 == 6:
                    s0 = sb * SBW
                    nc.vector.tensor_copy(kt[:, s0:s0 + 512], ps[0:64, 0:512])
                    nc.vector.tensor_copy(
                        kt[:, s0 + 512:s0 + SBW], ps[0:64, 512:SBW]
                    )
                    nc.vector.tensor_copy(
                        vt[:, s0:s0 + 512], ps[64:128, 0:512]
                    )
                    nc.vector.tensor_copy(
                        vt[:, s0 + 512:s0 + SBW], ps[64:128, 512:SBW]
                    )

            def emit_q(sb, k0, k1):
                """Chunks k0:k1 of sb's Q projection chain (sb 0 or 1)."""
                if k0 == 0:
                    kv_state[("q", sb)] = psmm.tile(
                        [128, HALF], F32, tag="mm", name=f"q_ps{sb}"
                    )
                ps = kv_state[("q", sb)]
                for k in range(k0, k1):
                    for c in range(HALF // MMW):
                        csl = slice(c * MMW, (c + 1) * MMW)
                        nc.tensor.matmul(
                            ps[:, csl],
                            wt_sb[:, k * 256 + 128:(k + 1) * 256],
                            xt_tiles[sb][:, k, c * MMW:(c + 1) * MMW],
                            start=(k == 0), stop=(k == 5),
                        )
                if k1 == 6:
                    nc.vector.tensor_copy(
                        qt[:, sb * SBW:(sb + 1) * SBW], ps[0:64, :]
                    )
                    kv_state.pop(("q", sb))

            def emit_vtrans(sb, i0, i1):
                """V natural tiles i0:i1 (of 8) for sb via PE transpose into
                the spent kv_ps tile (bf16 view; WAR on the kt/vt copies)."""
                ps16 = kv_state[sb][:].bitcast(BF16)
                for i in range(i0, i1):
                    t = sb * 8 + i
                    tsl = slice(i * 64, (i + 1) * 64)
                    nc.tensor.transpose(
                        ps16[:, tsl],
                        vt[:, t * 128:(t + 1) * 128],
                        identb[0:64, 0:64],
                    )
                    nc.vector.tensor_copy(v_sb[:, t, 0:64], ps16[:, tsl])
                if i1 == 8:
                    kv_state.pop(sb)

            def emit_vtrans_dma(sb):
                """V natural tiles for sb via the DMA xbar transpose engine
                (SP hwdge queue, idle after input issue). Only for sb>=2:
                earlier tiles are needed before the SP queue drains."""
                for i in range(8):
                    t = sb * 8 + i
                    nc.sync.dma_start_transpose(
                        out=v_sb[:, t, 0:64],
                        in_=vt[:, t * 128:(t + 1) * 128],
                    )
                kv_state.pop(sb, None)

            ats = {}
            pts = {}
            accs = {}

            def emit_scores(h, t):
                at = psmm.tile([128, HALF], F32, tag="mm")
                for c in range(HALF // MMW):
                    csl = slice(c * MMW, (c + 1) * MMW)
                    qsl = slice(h * HALF + c * MMW, h * HALF + (c + 1) * MMW)
                    nc.tensor.matmul(
                        at[:, csl],
                        kt[:, t * 128:(t + 1) * 128],
                        qt[:, qsl],
                        start=True, stop=True,
                    )
                pt = ptp.tile([128, HALF], BF16, tag="pt")
                nc.scalar.activation(
                    out=pt[:], in_=at[:],
                    func=mybir.ActivationFunctionType.Exp, bias=ebias[:],
                )
                pts[(h, t)] = pt

            def emit_av(h, t):
                pt = pts.pop((h, t))
                acc = accs[h]
                for c in range(HALF // MMW):
                    csl = slice(c * MMW, (c + 1) * MMW)
                    nc.tensor.matmul(
                        acc[:, csl],
                        v_sb[:, t, :],
                        pt[:, csl],
                        start=(t == 0), stop=(t == NKT - 1),
                        skip_group_check=True,
                    )

            def emit_out_dma(oc):
                nc.sync.dma_start(
                    out=out_d[:].rearrange("(t p) d -> p t d", p=128)[
                        :, 4 * oc:4 * (oc + 1), :],
                    in_=osb[:, 4 * oc:4 * (oc + 1), :],
                )

            def emit_fin(gblks):
                """Batch: transpose each 128-q block into one psum tile, one
                batched reciprocal, then per-block scale (avoids the PE-DVE
                ping-pong that stalled the tail)."""
                n = len(gblks)
                ot = psmm.tile([128, HALF], F32, tag="mm")
                for j, g in enumerate(gblks):
                    nc.tensor.transpose(
                        ot[:, 128 * j:128 * j + 65],
                        acc_sb[:, g * 128:(g + 1) * 128],
                        ident[0:65, 0:65],
                    )
                r = small.tile([128, 4], F32, tag="r")
                nc.vector.reciprocal(
                    r[:, 0:n],
                    ot[:].rearrange("p (j c) -> p j c", c=128)[:, 0:n, 64:65],
                )
                for j, g in enumerate(gblks):
                    nc.vector.tensor_scalar(
                        osb[:, g, :], ot[:, 128 * j:128 * j + 64], r[:, j:j + 1],
                        0.125,
                        op0=mybir.AluOpType.mult, op1=mybir.AluOpType.mult,
                    )

            # ---- pre-slot ramp: sb0 proj (kv then q), then V tiles 0:8 ----
            emit_kv(0, 0, 2)
            emit_kv(0, 2, 4)
            emit_kv(0, 4, 6)
            emit_q(0, 0, 6)
            emit_vtrans(0, 0, 8)
            accs[0] = psacc.tile([128, HALF], F32, tag="acc", name="acc0")

            # ---- slot stream -------------------------------------------
            # filler[i]: PE work emitted right after scores of slot i.
            # kv_sb must land before its slots (kt t: sb1 -> slots 8.., etc);
            # Vtrans sb before its AV slots; q1 before slot 32; all proj
            # windows sit in h0 so h1 slots have a free psmm buf for fins.
            filler = {
                0: [("kv", 1, 0, 3)], 1: [("kv", 1, 3, 6)],
                2: [("vt", 1, 0, 2)], 3: [("vt", 1, 2, 4)],
                4: [("vt", 1, 4, 6)], 5: [("vt", 1, 6, 8)],
                6: [("kv", 2, 0, 3)], 7: [("kv", 2, 3, 6)],
                8: [("vtd", 2)],
                12: [("kv", 3, 0, 3)], 13: [("kv", 3, 3, 6)],
                14: [("vtd", 3)],
                21: [("q", 1, 0, 2)], 22: [("q", 1, 2, 4)], 23: [("q", 1, 4, 6)],
                # h1: finalize h0 blocks two per slot-pair; out DMAs staggered
                34: [("fin", (0, 1))], 36: [("fin", (2, 3))],
                38: [("fin", (4, 5))], 39: [("odma", 0)],
                40: [("fin", (6, 7))], 41: [("odma", 1)],
            }

            slots = [(0, t) for t in range(NKT)] + [(1, t) for t in range(NKT)]
            for i, (h, t) in enumerate(slots):
                emit_scores(h, t)
                for f in filler.get(i, ()):
                    if f[0] == "kv":
                        emit_kv(f[1], f[2], f[3])
                    elif f[0] == "q":
                        emit_q(f[1], f[2], f[3])
                    elif f[0] == "vt":
                        emit_vtrans(f[1], f[2], f[3])
                    elif f[0] == "vtd":
                        emit_vtrans_dma(f[1])
                    elif f[0] == "fin":
                        emit_fin(f[1])
                    elif f[0] == "odma":
                        emit_out_dma(f[1])
                if i > 0:
                    ph, pt_ = slots[i - 1]
                    emit_av(ph, pt_)
                    if (ph, pt_) == (0, NKT - 1):
                        # h0 accumulation complete: stage and swap acc
                        nc.vector.tensor_copy(
                            acc_sb[:, 0:512], accs[0][0:65, 0:512]
                        )
                        nc.vector.tensor_copy(
                            acc_sb[:, 512:HALF], accs[0][0:65, 512:HALF]
                        )
                        accs[1] = psacc.tile(
                            [128, HALF], F32, tag="acc", name="acc1"
                        )

            # ---- tail: last AV split so each 512-q chunk stages early ----
            fpt = pts.pop((1, NKT - 1))
            for c in range(2):
                csl = slice(c * 512, (c + 1) * 512)
                nc.tensor.matmul(
                    accs[1][:, csl], v_sb[:, NKT - 1, :], fpt[:, csl],
                    start=False, stop=True, skip_group_check=True,
                )
                for cc in range(2):
                    o = c * 512 + cc * 256
                    nc.vector.tensor_copy(
                        acc_sb[:, HALF + o:HALF + o + 256],
                        accs[1][0:65, c * 512 + cc * 256:c * 512 + (cc + 1) * 256],
                    )
                    g0 = 8 + 2 * (2 * c + cc)
                    emit_fin((g0, g0 + 1))
                emit_out_dma(2 + c)

    nc.finalize()
    return nc


def _get_nc():
    global _NC_CACHE
    if _NC_CACHE is None:
        _NC_CACHE = _build()
    return _NC_CACHE


def kernel(x, W, _trace=False):
    global LAST_RESULTS
    x = np.ascontiguousarray(np.asarray(x), dtype=np.float32)
    W = np.ascontiguousarray(np.asarray(W), dtype=np.float32)
    assert x.shape == (B, S, D) and W.shape == (3 * DH, D)

    # wt cols per chunk k: [K rows | V rows | Q rows | zero pad]
    wtf = np.concatenate(
        [W[DH:2 * DH], W[2 * DH:], W[:DH], np.zeros((DH, D), np.float32)],
        axis=0,
    ).T
    wt = np.ascontiguousarray(
        wtf.reshape(6, 128, 256).transpose(1, 0, 2).reshape(128, 6 * 256)
    ).astype(np.float16)

    in_maps = []
    for c in range(8):
        b, qh = divmod(c, 2)
        xtb = x[b].T.astype(np.float16)  # [768, 4096]
        if qh:
            xtc = np.ascontiguousarray(
                np.concatenate([xtb[:, QN:], xtb[:, :QN]], axis=1)
            )
        else:
            xtc = np.ascontiguousarray(xtb)
        in_maps.append({"xt": xtc, "wt": wt})

    nc = _get_nc()
    res = run_bass_kernel_spmd(nc, in_maps, list(range(8)), trace=_trace)
    LAST_RESULTS = res

    out = np.empty((B, S, DH), np.float32)
    for c in range(8):
        b, qh = divmod(c, 2)
        out[b, qh * QN:(qh + 1) * QN] = res.results[c]["out"]
    return out
